# revision 4
# baseline (speedup 1.0000x reference)
"""Trainium2 Bass kernel for LocalDenseConv2D + BatchNorm + PReLU (v13).

Conv as K=128 x M=128 bf16 matmuls (kernel_v3 docstring).  v5 scheduling
refinements from the v4 trace:
  - slots reordered (A_p -> q=2p, E_p -> q=2p+1) so DMA batches stream in
    exact consumption order; weight/x DMAs batched per pair and issued
    ahead of everything else on the sync queue (consts go via scalar)
  - no warm-up collective (measured: no effect on the real one)
  - global stats merge via tiny masked f32r matmuls on the (idle) PE:
    one contiguous DMA of the gathered table, 8 accumulating N=2
    matmuls reduce over (core, half), 1 broadcast matmul replicates
    per-channel sums to both partition halves.  Replaces the 2048x8B
    strided gather + triple-build (was ~6us of tail).
  - BN-apply: pairs 0-2 on scalar engine, pair 3 as 4-op prelu on the
    vector engine; all stores on the sync queue
"""
import sys
import numpy as np

if '/opt/trn_rl_repo' not in sys.path:
    sys.path.insert(0, '/opt/trn_rl_repo')

import concourse.bass as bass
import concourse.bacc as bacc
import concourse.mybir as mybir
import concourse.tile as tile
from concourse.bass_utils import run_bass_kernel_spmd

F32 = mybir.dt.float32
F32R = mybir.dt.float32r
BF16 = mybir.dt.bfloat16
AF = mybir.ActivationFunctionType
ALU = mybir.AluOpType

B, IN_C, L, T = 8, 64, 64, 256
OUT_C = 64
NCORES = 8
L_LOC = L // NCORES          # 8 out rows per core
NPAIR = L_LOC // 2           # 4 row pairs
SLAB = L_LOC + 2             # 10 slab rows incl. halo
TP = T + 2                   # padded t
EPS = 1e-5
N_LOC = NPAIR * 4 * 512      # elems per partition per core = 8192
N_GLOB = B * L * T           # 131072

_cache = {}


def _build():
    nc = bacc.Bacc("TRN2", target_bir_lowering=False, debug=False,
                   num_devices=NCORES)
    xq = nc.dram_tensor("xq", [128, 8, B, TP], BF16, kind="ExternalInput")
    wq = nc.dram_tensor("wq", [128, 8, 3, 128], BF16, kind="ExternalInput")
    bb = nc.dram_tensor("bb", [128, NPAIR], F32, kind="ExternalInput")
    gr = nc.dram_tensor("gr", [128, 1], F32, kind="ExternalInput")
    er = nc.dram_tensor("er", [128, 1], F32, kind="ExternalInput")
    ar = nc.dram_tensor("ar", [128, 1], F32, kind="ExternalInput")
    m8 = nc.dram_tensor("m8", [128, 8, 128], F32, kind="ExternalInput")
    yo = nc.dram_tensor("yo", [NPAIR, 128, 4, 512], BF16, kind="ExternalOutput")

    cc_in = nc.dram_tensor("cc_in", [128, 2], F32)
    cc_out = nc.dram_tensor("cc_out", [NCORES * 128, 2], F32,
                            addr_space="Shared")

    with tile.TileContext(nc) as tc:
        with (
            tc.tile_pool(name="const", bufs=1) as cpool,
            tc.tile_pool(name="fp", bufs=3) as fpool,
            tc.tile_pool(name="ps", bufs=8, space="PSUM") as ppool,
        ):
            wt = cpool.tile([128, 8, 3, 128], BF16)
            xt = cpool.tile([128, 8, B, TP], BF16)
            bt = cpool.tile([128, NPAIR], F32)
            gt = cpool.tile([128, 1], F32)
            et = cpool.tile([128, 1], F32)
            at = cpool.tile([128, 1], F32)
            m8t = cpool.tile([128, 8, 128], F32)
            ot = cpool.tile([128, NPAIR, 4, 512], BF16)
            stats = cpool.tile([128, NPAIR, 4, 6], F32)
            epst = cpool.tile([128, 1], F32)
            warm = cpool.tile([128, 1], F32)
            oma = cpool.tile([128, 1], F32)

            # weights + x on the sync queue, in consumption order; the
            # first micro-batch (pair-0 slots, batches 0-1) unblocks the
            # first chunk of matmuls early
            nc.sync.dma_start(wt[:, 0:2], wq.ap()[:, 0:2])
            nc.sync.dma_start(xt[:, 0:2, 0:2], xq.ap()[:, 0:2, 0:2])
            nc.sync.dma_start(xt[:, 0:2, 2:4], xq.ap()[:, 0:2, 2:4])
            nc.sync.dma_start(xt[:, 0:2, 4:6], xq.ap()[:, 0:2, 4:6])
            nc.sync.dma_start(xt[:, 0:2, 6:8], xq.ap()[:, 0:2, 6:8])
            nc.sync.dma_start(wt[:, 2:8], wq.ap()[:, 2:8])
            nc.sync.dma_start(xt[:, 2:4, 0:2], xq.ap()[:, 2:4, 0:2])
            nc.sync.dma_start(xt[:, 2:4, 2:8], xq.ap()[:, 2:4, 2:8])
            nc.sync.dma_start(xt[:, 4:6], xq.ap()[:, 4:6])
            nc.sync.dma_start(xt[:, 6:8], xq.ap()[:, 6:8])
            # small consts via the scalar queue
            nc.scalar.dma_start(bt[:], bb.ap())
            nc.scalar.dma_start(gt[:], gr.ap())
            nc.scalar.dma_start(et[:], er.ap())
            nc.scalar.dma_start(at[:], ar.ap())
            nc.scalar.dma_start(m8t[:], m8.ap())
            nc.vector.memset(epst[:], EPS)
            nc.vector.memset(oma[:], 1.0)
            nc.vector.tensor_tensor(oma[:], oma[:], at[:], ALU.subtract)
            nc.scalar.activation(warm[:], epst[:], AF.Sqrt, bias=epst[:])

            # ---- conv: one PSUM bank per chunk so stats/copies never
            # serialize the next chunk's matmuls (tile-granular WAR) ----
            for p in range(NPAIR):
                for n in range(4):
                    pp = ppool.tile([128, 512], F32, tag="pp")
                    for dj in range(3):
                        for kind in range(2):      # 0 = dense, 1 = edge
                            q = 2 * p + kind
                            nc.tensor.matmul(
                                pp[:],
                                wt[:, q, dj, :],
                                xt[:, q, 2 * n:2 * n + 2, dj:dj + T],
                                start=(dj == 0 and kind == 0),
                                stop=(dj == 2 and kind == 1))
                    nc.vector.bn_stats(stats[:, p, n, :], pp[:])
                    nc.scalar.activation(
                        ot[:, p, n, :], pp[:],
                        AF.Identity, bias=bt[:, p:p + 1])

            # ---- local aggregate -> (mean, E[x^2]) -> collective ----
            loc = cpool.tile([128, 2], F32)
            m2s = cpool.tile([128, 1], F32)
            nc.vector.bn_aggr(loc[:],
                              stats[:].rearrange("p a b c -> p (a b c)"))
            nc.vector.tensor_tensor(m2s[:], loc[:, 0:1], loc[:, 0:1],
                                    ALU.mult)
            nc.vector.tensor_tensor(loc[:, 1:2], loc[:, 1:2], m2s[:],
                                    ALU.add)
            nc.sync.dma_start(cc_in.ap(), loc[:])
            nc.gpsimd.collective_compute(
                "AllGather", ALU.bypass,
                replica_groups=[list(range(NCORES))],
                ins=[cc_in[:]], outs=[cc_out[:]])

            # ---- global merge on the PE: one contiguous read, then
            # masked f32r matmuls reduce over (core, half) + broadcast ----
            gd = cpool.tile([128, 16], F32)
            nc.sync.dma_start(gd[:], cc_out.ap().rearrange(
                "(p j) s -> p (j s)", p=128))
            mg = ppool.tile([128, 512], F32, tag="pp")
            for j in range(8):
                nc.tensor.matmul(mg[:, 0:2], m8t[:, j, :],
                                 gd[:, 2 * j:2 * j + 2],
                                 start=(j == 0), stop=(j == 7))


            # scale = gamma / sqrt(var + eps); shift = beta - mean * scale
            var = cpool.tile([128, 1], F32)
            stdt = cpool.tile([128, 1], F32)
            rstd = cpool.tile([128, 1], F32)
            sca = cpool.tile([128, 1], F32)
            shi = cpool.tile([128, 1], F32)
            nc.scalar.square(stdt[:], mg[:, 0:1])
            nc.vector.tensor_tensor(var[:], mg[:, 1:2], stdt[:],
                                    ALU.subtract)
            nc.scalar.activation(stdt[:], var[:], AF.Sqrt, bias=epst[:])
            nc.vector.reciprocal(rstd[:], stdt[:])
            nc.vector.tensor_tensor(sca[:], gt[:], rstd[:], ALU.mult)
            nc.vector.tensor_tensor(shi[:], mg[:, 0:1], sca[:], ALU.mult)
            nc.vector.tensor_tensor(shi[:], et[:], shi[:], ALU.subtract)

            # ---- fused BN-apply + PReLU + store ----
            def dve_prelu(dst, src):
                z = cpool.tile(list(dst.shape), BF16)
                rl = cpool.tile(list(dst.shape), BF16)
                nc.vector.tensor_scalar(z[:], src, sca[:], shi[:],
                                        ALU.mult, ALU.add)
                nc.vector.tensor_scalar(rl[:], z[:], 0.0, oma[:],
                                        ALU.max, ALU.mult)
                nc.vector.tensor_scalar(z[:], z[:], at[:], None, ALU.mult)
                nc.vector.tensor_tensor(dst, rl[:], z[:], ALU.add)

            fo2 = cpool.tile([128, 4096], BF16)
            nc.scalar.activation(
                fo2[:], ot[:, 0:2].rearrange("p a b c -> p (a b c)"),
                AF.Prelu, bias=shi[:], scale=sca[:], alpha=at[:])
            nc.sync.dma_start(
                yo.ap()[0].rearrange("p a b -> p (a b)"), fo2[:, 0:2048])
            nc.sync.dma_start(
                yo.ap()[1].rearrange("p a b -> p (a b)"), fo2[:, 2048:4096])
            # pair 3 fully + second half of pair 2 on the vector engine,
            # first half of pair 2 on the scalar engine (parallel finish)
            f3 = cpool.tile([128, 2048], BF16)
            dve_prelu(f3[:], ot[:, 3].rearrange("p a b -> p (a b)"))
            nc.sync.dma_start(
                yo.ap()[3].rearrange("p a b -> p (a b)"), f3[:])
            f2 = cpool.tile([128, 2048], BF16)
            o2 = ot[:, 2].rearrange("p a b -> p (a b)")
            nc.scalar.activation(
                f2[:, 0:1024], o2[:, 0:1024],
                AF.Prelu, bias=shi[:], scale=sca[:], alpha=at[:])
            dve_prelu(f2[:, 1024:2048], o2[:, 1024:2048])
            nc.sync.dma_start(
                yo.ap()[2].rearrange("p a b -> p (a b)"), f2[:])
    nc.compile()
    return nc


def _prep(x, weight, bias, gamma, beta, alpha):
    """Host-side shard + relayout into per-core input maps."""
    import ml_dtypes
    bf16 = ml_dtypes.bfloat16

    xpad = np.zeros((B, IN_C, L + 2, TP), np.float32)
    xpad[:, :, 1:L + 1, 1:T + 1] = x
    w4 = weight.reshape(IN_C, 3, 3, OUT_C, L)   # [ci, di, dj, c, l]

    gamma2 = np.tile(gamma, 2).reshape(128, 1).astype(np.float32)
    beta2 = np.tile(beta, 2).reshape(128, 1).astype(np.float32)
    alpha2 = np.full((128, 1), float(alpha[0]), np.float32)

    # merge masks: partition p of the gathered table holds rows
    # 8p..8p+7 of cc_out, i.e. channels 8*(p%8)+j for half h=(p//8)%2,
    # core r=p//16.  m8[p, j, c] selects channel 8*(p%8)+j.
    m8 = np.zeros((128, 8, 128), np.float32)
    for p in range(128):
        for j in range(8):
            c = 8 * (p % 8) + j
            m8[p, j, c] = 1.0 / 16.0
            m8[p, j, 64 + c] = 1.0 / 16.0

    in_maps = []
    for r in range(NCORES):
        l0 = r * L_LOC
        slab = xpad[:, :, l0:l0 + SLAB, :].transpose(1, 2, 0, 3)  # ci,s,b,t
        xq = np.empty((128, 8, B, TP), np.float32)
        for p in range(NPAIR):
            xq[0:64, 2 * p] = slab[:, 2 * p + 1]        # A up
            xq[64:128, 2 * p] = slab[:, 2 * p + 2]      # A low
            xq[0:64, 2 * p + 1] = slab[:, 2 * p]        # E up
            xq[64:128, 2 * p + 1] = slab[:, 2 * p + 3]  # E low

        wqh = np.zeros((128, 8, 3, 128), np.float32)
        for p in range(NPAIR):
            for dj in range(3):
                qa, qe = 2 * p, 2 * p + 1
                wqh[0:64, qa, dj, 0:64] = w4[:, 1, dj, :, l0 + 2 * p]
                wqh[0:64, qa, dj, 64:128] = w4[:, 0, dj, :, l0 + 2 * p + 1]
                wqh[64:128, qa, dj, 0:64] = w4[:, 2, dj, :, l0 + 2 * p]
                wqh[64:128, qa, dj, 64:128] = w4[:, 1, dj, :, l0 + 2 * p + 1]
                wqh[0:64, qe, dj, 0:64] = w4[:, 0, dj, :, l0 + 2 * p]
                wqh[64:128, qe, dj, 64:128] = w4[:, 2, dj, :, l0 + 2 * p + 1]

        bbh = np.empty((128, NPAIR), np.float32)
        for p in range(NPAIR):
            bbh[0:64, p] = bias[:, l0 + 2 * p]
            bbh[64:128, p] = bias[:, l0 + 2 * p + 1]

        in_maps.append({
            "xq": xq.astype(bf16), "wq": wqh.astype(bf16), "bb": bbh,
            "gr": gamma2, "er": beta2, "ar": alpha2,
            "m8": m8,
        })
    return in_maps


def kernel(x, weight, bias, gamma, beta, alpha, trace=False,
           trace_all=False):
    x = np.asarray(x, np.float32)
    weight = np.asarray(weight, np.float32)
    bias = np.asarray(bias, np.float32)
    gamma = np.asarray(gamma, np.float32)
    beta = np.asarray(beta, np.float32)
    alpha = np.asarray(alpha, np.float32)

    if "nc" not in _cache:
        _cache["nc"] = _build()
    nc = _cache["nc"]
    in_maps = _prep(x, weight, bias, gamma, beta, alpha)
    kw = {}
    if trace_all:
        kw["trace_cores"] = list(range(NCORES))
    res = run_bass_kernel_spmd(nc, in_maps, list(range(NCORES)),
                               trace=trace, **kw)
    kernel._last = res

    out = np.empty((B, OUT_C, L, T), np.float32)
    for r in range(NCORES):
        yo = np.asarray(res.results[r]["yo"]).astype(np.float32)
        # yo[p, rr*64+c, n, b2*256+t] -> out[2n+b2, c, l0+2p+rr, t]
        yo = yo.reshape(NPAIR, 2, 64, 4, 2, T).transpose(3, 4, 2, 0, 1, 5)
        out[:, :, r * L_LOC:(r + 1) * L_LOC, :] = yo.reshape(B, 64, L_LOC, T)
    return out


# revision 5
# speedup vs baseline: 1.0236x; 1.0236x over previous
"""Trainium2 Bass kernel for LocalDenseConv2D + BatchNorm + PReLU (v14).

Conv as K=128 x M=128 bf16 matmuls (kernel_v3 docstring).  v5 scheduling
refinements from the v4 trace:
  - slots reordered (A_p -> q=2p, E_p -> q=2p+1) so DMA batches stream in
    exact consumption order; weight/x DMAs batched per pair and issued
    ahead of everything else on the sync queue (consts go via scalar)
  - no warm-up collective (measured: no effect on the real one)
  - global stats merge via tiny masked f32r matmuls on the (idle) PE:
    one contiguous DMA of the gathered table, 8 accumulating N=2
    matmuls reduce over (core, half), 1 broadcast matmul replicates
    per-channel sums to both partition halves.  Replaces the 2048x8B
    strided gather + triple-build (was ~6us of tail).
  - BN-apply: pairs 0-2 on scalar engine, pair 3 as 4-op prelu on the
    vector engine; all stores on the sync queue
"""
import sys
import numpy as np

if '/opt/trn_rl_repo' not in sys.path:
    sys.path.insert(0, '/opt/trn_rl_repo')

import concourse.bass as bass
import concourse.bacc as bacc
import concourse.mybir as mybir
import concourse.tile as tile
from concourse.bass_utils import run_bass_kernel_spmd

F32 = mybir.dt.float32
F32R = mybir.dt.float32r
BF16 = mybir.dt.bfloat16
AF = mybir.ActivationFunctionType
ALU = mybir.AluOpType

B, IN_C, L, T = 8, 64, 64, 256
OUT_C = 64
NCORES = 8
L_LOC = L // NCORES          # 8 out rows per core
NPAIR = L_LOC // 2           # 4 row pairs
SLAB = L_LOC + 2             # 10 slab rows incl. halo
TP = T + 2                   # padded t
EPS = 1e-5
N_LOC = NPAIR * 4 * 512      # elems per partition per core = 8192
N_GLOB = B * L * T           # 131072

_cache = {}


def _build():
    nc = bacc.Bacc("TRN2", target_bir_lowering=False, debug=False,
                   num_devices=NCORES)
    xq = nc.dram_tensor("xq", [128, 8, B, TP], BF16, kind="ExternalInput")
    wq = nc.dram_tensor("wq", [128, 8, 3, 128], BF16, kind="ExternalInput")
    bb = nc.dram_tensor("bb", [128, NPAIR], F32, kind="ExternalInput")
    gr = nc.dram_tensor("gr", [128, 1], F32, kind="ExternalInput")
    er = nc.dram_tensor("er", [128, 1], F32, kind="ExternalInput")
    ar = nc.dram_tensor("ar", [128, 1], F32, kind="ExternalInput")
    m8 = nc.dram_tensor("m8", [128, 8, 128], F32, kind="ExternalInput")
    yo = nc.dram_tensor("yo", [NPAIR, 128, 4, 512], BF16, kind="ExternalOutput")

    cc_in = nc.dram_tensor("cc_in", [128, 2], F32)
    cc_out = nc.dram_tensor("cc_out", [NCORES * 128, 2], F32,
                            addr_space="Shared")

    with tile.TileContext(nc) as tc:
        with (
            tc.tile_pool(name="const", bufs=1) as cpool,
            tc.tile_pool(name="fp", bufs=3) as fpool,
            tc.tile_pool(name="ps", bufs=8, space="PSUM") as ppool,
        ):
            wt = cpool.tile([128, 8, 3, 128], BF16)
            xt = cpool.tile([128, 8, B, TP], BF16)
            bt = cpool.tile([128, NPAIR], F32)
            gt = cpool.tile([128, 1], F32)
            et = cpool.tile([128, 1], F32)
            at = cpool.tile([128, 1], F32)
            m8t = cpool.tile([128, 8, 128], F32)
            ot = cpool.tile([128, NPAIR, 4, 512], BF16)
            stats = cpool.tile([128, NPAIR, 4, 6], F32)
            epst = cpool.tile([128, 1], F32)
            warm = cpool.tile([128, 1], F32)
            oma = cpool.tile([128, 1], F32)

            # weights + x on the sync queue, in consumption order; the
            # first micro-batch (pair-0 slots, batches 0-1) unblocks the
            # first chunk of matmuls early
            nc.sync.dma_start(wt[:, 0:2], wq.ap()[:, 0:2])
            nc.sync.dma_start(xt[:, 0:2, 0:2], xq.ap()[:, 0:2, 0:2])
            nc.sync.dma_start(xt[:, 0:2, 2:4], xq.ap()[:, 0:2, 2:4])
            nc.sync.dma_start(xt[:, 0:2, 4:6], xq.ap()[:, 0:2, 4:6])
            nc.sync.dma_start(xt[:, 0:2, 6:8], xq.ap()[:, 0:2, 6:8])
            nc.sync.dma_start(wt[:, 2:8], wq.ap()[:, 2:8])
            nc.sync.dma_start(xt[:, 2:4, 0:2], xq.ap()[:, 2:4, 0:2])
            nc.sync.dma_start(xt[:, 2:4, 2:8], xq.ap()[:, 2:4, 2:8])
            nc.sync.dma_start(xt[:, 4:6], xq.ap()[:, 4:6])
            nc.sync.dma_start(xt[:, 6:8], xq.ap()[:, 6:8])
            # small consts via the scalar queue
            nc.scalar.dma_start(bt[:], bb.ap())
            nc.scalar.dma_start(gt[:], gr.ap())
            nc.scalar.dma_start(et[:], er.ap())
            nc.scalar.dma_start(at[:], ar.ap())
            nc.scalar.dma_start(m8t[:], m8.ap())
            nc.vector.memset(epst[:], EPS)
            nc.vector.memset(oma[:], 1.0)
            nc.vector.tensor_tensor(oma[:], oma[:], at[:], ALU.subtract)
            nc.scalar.activation(warm[:], epst[:], AF.Sqrt, bias=epst[:])

            # ---- conv: one PSUM bank per chunk so stats/copies never
            # serialize the next chunk's matmuls (tile-granular WAR) ----
            for p in range(NPAIR):
                for n in range(4):
                    pp = ppool.tile([128, 512], F32, tag="pp")
                    for dj in range(3):
                        for kind in range(2):      # 0 = dense, 1 = edge
                            q = 2 * p + kind
                            nc.tensor.matmul(
                                pp[:],
                                wt[:, q, dj, :],
                                xt[:, q, 2 * n:2 * n + 2, dj:dj + T],
                                start=(dj == 0 and kind == 0),
                                stop=(dj == 2 and kind == 1))
                    nc.vector.bn_stats(stats[:, p, n, :], pp[:])
                    nc.scalar.activation(
                        ot[:, p, n, :], pp[:],
                        AF.Identity, bias=bt[:, p:p + 1])

            # ---- local aggregate -> (mean, E[x^2]) -> collective ----
            loc = cpool.tile([128, 2], F32)
            m2s = cpool.tile([128, 1], F32)
            nc.vector.bn_aggr(loc[:],
                              stats[:].rearrange("p a b c -> p (a b c)"))
            nc.vector.tensor_tensor(m2s[:], loc[:, 0:1], loc[:, 0:1],
                                    ALU.mult)
            nc.vector.tensor_tensor(loc[:, 1:2], loc[:, 1:2], m2s[:],
                                    ALU.add)
            nc.sync.dma_start(cc_in.ap(), loc[:])
            nc.gpsimd.collective_compute(
                "AllGather", ALU.bypass,
                replica_groups=[list(range(NCORES))],
                ins=[cc_in[:]], outs=[cc_out[:]])

            # ---- global merge on the PE: one contiguous read, then
            # masked f32r matmuls reduce over (core, half) + broadcast ----
            gd = cpool.tile([128, 16], F32)
            nc.sync.dma_start(gd[:], cc_out.ap().rearrange(
                "(p j) s -> p (j s)", p=128))
            mg = ppool.tile([128, 512], F32, tag="pp")
            for j in range(8):
                nc.tensor.matmul(mg[:, 0:2], m8t[:, j, :],
                                 gd[:, 2 * j:2 * j + 2],
                                 start=(j == 0), stop=(j == 7))


            # scale = gamma / sqrt(var + eps); shift = beta - mean * scale
            var = cpool.tile([128, 1], F32)
            stdt = cpool.tile([128, 1], F32)
            rstd = cpool.tile([128, 1], F32)
            sca = cpool.tile([128, 1], F32)
            shi = cpool.tile([128, 1], F32)
            nc.scalar.square(stdt[:], mg[:, 0:1])
            nc.vector.tensor_tensor(var[:], mg[:, 1:2], stdt[:],
                                    ALU.subtract)
            nc.scalar.activation(stdt[:], var[:], AF.Sqrt, bias=epst[:])
            nc.vector.reciprocal(rstd[:], stdt[:])
            nc.vector.tensor_tensor(sca[:], gt[:], rstd[:], ALU.mult)
            nc.vector.tensor_tensor(shi[:], mg[:, 0:1], sca[:], ALU.mult)
            nc.vector.tensor_tensor(shi[:], et[:], shi[:], ALU.subtract)

            # ---- fused BN-apply + PReLU + store ----
            def dve_prelu(dst, src):
                z = cpool.tile(list(dst.shape), BF16)
                rl = cpool.tile(list(dst.shape), BF16)
                nc.vector.tensor_scalar(z[:], src, sca[:], shi[:],
                                        ALU.mult, ALU.add)
                nc.vector.tensor_scalar(rl[:], z[:], 0.0, oma[:],
                                        ALU.max, ALU.mult)
                nc.vector.tensor_scalar(z[:], z[:], at[:], None, ALU.mult)
                nc.vector.tensor_tensor(dst, rl[:], z[:], ALU.add)

            fo2 = cpool.tile([128, 4096], BF16)
            nc.scalar.activation(
                fo2[:], ot[:, 0:2].rearrange("p a b c -> p (a b c)"),
                AF.Prelu, bias=shi[:], scale=sca[:], alpha=at[:])
            nc.sync.dma_start(
                yo.ap()[0].rearrange("p a b -> p (a b)"), fo2[:, 0:2048])
            nc.sync.dma_start(
                yo.ap()[1].rearrange("p a b -> p (a b)"), fo2[:, 2048:4096])
            # pair 3 fully + second half of pair 2 on the vector engine,
            # first half of pair 2 on the scalar engine (parallel finish)
            f3 = cpool.tile([128, 2048], BF16)
            dve_prelu(f3[:], ot[:, 3].rearrange("p a b -> p (a b)"))
            nc.sync.dma_start(
                yo.ap()[3].rearrange("p a b -> p (a b)"), f3[:])
            f2 = cpool.tile([128, 2048], BF16)
            o2 = ot[:, 2].rearrange("p a b -> p (a b)")
            nc.scalar.activation(
                f2[:, 0:1280], o2[:, 0:1280],
                AF.Prelu, bias=shi[:], scale=sca[:], alpha=at[:])
            dve_prelu(f2[:, 1280:2048], o2[:, 1280:2048])
            nc.sync.dma_start(
                yo.ap()[2].rearrange("p a b -> p (a b)"), f2[:])
    nc.compile()
    return nc


def _prep(x, weight, bias, gamma, beta, alpha):
    """Host-side shard + relayout into per-core input maps."""
    import ml_dtypes
    bf16 = ml_dtypes.bfloat16

    xpad = np.zeros((B, IN_C, L + 2, TP), np.float32)
    xpad[:, :, 1:L + 1, 1:T + 1] = x
    w4 = weight.reshape(IN_C, 3, 3, OUT_C, L)   # [ci, di, dj, c, l]

    gamma2 = np.tile(gamma, 2).reshape(128, 1).astype(np.float32)
    beta2 = np.tile(beta, 2).reshape(128, 1).astype(np.float32)
    alpha2 = np.full((128, 1), float(alpha[0]), np.float32)

    # merge masks: partition p of the gathered table holds rows
    # 8p..8p+7 of cc_out, i.e. channels 8*(p%8)+j for half h=(p//8)%2,
    # core r=p//16.  m8[p, j, c] selects channel 8*(p%8)+j.
    m8 = np.zeros((128, 8, 128), np.float32)
    for p in range(128):
        for j in range(8):
            c = 8 * (p % 8) + j
            m8[p, j, c] = 1.0 / 16.0
            m8[p, j, 64 + c] = 1.0 / 16.0

    in_maps = []
    for r in range(NCORES):
        l0 = r * L_LOC
        slab = xpad[:, :, l0:l0 + SLAB, :].transpose(1, 2, 0, 3)  # ci,s,b,t
        xq = np.empty((128, 8, B, TP), np.float32)
        for p in range(NPAIR):
            xq[0:64, 2 * p] = slab[:, 2 * p + 1]        # A up
            xq[64:128, 2 * p] = slab[:, 2 * p + 2]      # A low
            xq[0:64, 2 * p + 1] = slab[:, 2 * p]        # E up
            xq[64:128, 2 * p + 1] = slab[:, 2 * p + 3]  # E low

        wqh = np.zeros((128, 8, 3, 128), np.float32)
        for p in range(NPAIR):
            for dj in range(3):
                qa, qe = 2 * p, 2 * p + 1
                wqh[0:64, qa, dj, 0:64] = w4[:, 1, dj, :, l0 + 2 * p]
                wqh[0:64, qa, dj, 64:128] = w4[:, 0, dj, :, l0 + 2 * p + 1]
                wqh[64:128, qa, dj, 0:64] = w4[:, 2, dj, :, l0 + 2 * p]
                wqh[64:128, qa, dj, 64:128] = w4[:, 1, dj, :, l0 + 2 * p + 1]
                wqh[0:64, qe, dj, 0:64] = w4[:, 0, dj, :, l0 + 2 * p]
                wqh[64:128, qe, dj, 64:128] = w4[:, 2, dj, :, l0 + 2 * p + 1]

        bbh = np.empty((128, NPAIR), np.float32)
        for p in range(NPAIR):
            bbh[0:64, p] = bias[:, l0 + 2 * p]
            bbh[64:128, p] = bias[:, l0 + 2 * p + 1]

        in_maps.append({
            "xq": xq.astype(bf16), "wq": wqh.astype(bf16), "bb": bbh,
            "gr": gamma2, "er": beta2, "ar": alpha2,
            "m8": m8,
        })
    return in_maps


def kernel(x, weight, bias, gamma, beta, alpha, trace=False,
           trace_all=False):
    x = np.asarray(x, np.float32)
    weight = np.asarray(weight, np.float32)
    bias = np.asarray(bias, np.float32)
    gamma = np.asarray(gamma, np.float32)
    beta = np.asarray(beta, np.float32)
    alpha = np.asarray(alpha, np.float32)

    if "nc" not in _cache:
        _cache["nc"] = _build()
    nc = _cache["nc"]
    in_maps = _prep(x, weight, bias, gamma, beta, alpha)
    kw = {}
    if trace_all:
        kw["trace_cores"] = list(range(NCORES))
    res = run_bass_kernel_spmd(nc, in_maps, list(range(NCORES)),
                               trace=trace, **kw)
    kernel._last = res

    out = np.empty((B, OUT_C, L, T), np.float32)
    for r in range(NCORES):
        yo = np.asarray(res.results[r]["yo"]).astype(np.float32)
        # yo[p, rr*64+c, n, b2*256+t] -> out[2n+b2, c, l0+2p+rr, t]
        yo = yo.reshape(NPAIR, 2, 64, 4, 2, T).transpose(3, 4, 2, 0, 1, 5)
        out[:, :, r * L_LOC:(r + 1) * L_LOC, :] = yo.reshape(B, 64, L_LOC, T)
    return out
